# revision 24
# baseline (speedup 1.0000x reference)
"""MinGRU on Trainium2 (Bass/Tile), data-parallel over batch on 8 NeuronCores.

Math (per batch element, per hidden channel):
    k_z = x @ W_z.T + b_z
    k_h = x @ W_h.T + b_h
    a   = sigmoid(-k_z)                  # = exp(log_coeffs) in the reference
    z   = sigmoid(k_z) = 1 - a
    g(u)= u + 0.5 if u >= 0 else sigmoid(u)
    v   = z * g(k_h)                     # = exp(log_values[1:])
    h_t = a_t * h_{t-1} + v_t,  h_init = g(h_0)        (t = 1..T)
Output is h_1..h_T, shape [B, T, H].

Device layout: each core gets one batch element. Hidden dim H lives on
SBUF partitions (8 tiles of 128), time T on the free dim, so the
recurrence maps to the DVE TensorTensorScan instruction (fp32 state).

Precision/throughput split (validated by CPU emulation, rel_err 1.5e-2
vs 2e-2 gate):
  - k_z runs in fp8(e4m3) with DoubleRow perf mode (2 contraction rows
    per PE cell -> 2x matmul rate). Errors in k_z only reach h through
    sigmoid' <= 1/4, so fp8 noise is tolerable there.
  - k_h runs in bf16 (g has slope 1; fp8 here fails the error gate).
  - Post-PSUM work uses bf16 tiles and only tensor_scalar (4x DVE rate)
    / tensor_tensor (2x) / scan ops; the 1x-rate STT form is avoided:
        gm = min(s, 0.5); g = gm + r; z = (a-1)*(-1); v = z*g
        h = scan(a, v): state = a*state + v
    Each chunk's scan seeds directly from the previous h tile (no carry
    copies). fp8 operands are pre-scaled (x*32, W_z*4096); the 2^-17
    descale folds into the ACT scale. Output is bf16, upcast on host.

DMA rings: gpsimd carries wz8 + fp8 x chunks, scalar carries consts +
bf16 x chunks, sync carries wh (queued before any store) + the output
stores. Stores wait on the scan, so no load may queue behind them.
DMA-completion semaphore slots rotate per queue and only the FIRST
queue-consumer of a DMA gets a wait instruction, so each chunk's
loads are emitted immediately before their first consumers (chunk
boundaries). Chunk 0 runs all 8 k_z PE groups (which need only 1.5 MB
of fp8 data) before any k_h group so the PE starts ~7us in while
wh/x16 stream.
"""

import numpy as np
from contextlib import ExitStack

import concourse.bass as bass
import concourse.tile as tile
from concourse import bacc, mybir
from concourse.bass_utils import run_bass_kernel_spmd

B, T, I, H = 8, 4096, 1024, 1024
P = 128           # SBUF partitions
TC = 1024         # max T chunk for the PSUM/ACT/DVE stage (2 PSUM banks)
MN = 512          # matmul moving free dim (one PSUM bank of fp32)
CHUNKS = [768, 1024, 1024, 1024, 256]
assert sum(CHUNKS) == T
NI, NH = I // P, H // P
NQ = NI // 2      # fp8 DoubleRow contraction pairs
NT = len(CHUNKS)
F32 = mybir.dt.float32
BF16 = mybir.dt.bfloat16
FP8 = mybir.dt.float8e4
import ml_dtypes
BF16_NP = ml_dtypes.bfloat16
FP8_NP = ml_dtypes.float8_e4m3
XS = 32.0         # host-side fp8 scale on x
WS = 4096.0       # host-side fp8 scale on W_z
DESCALE = 1.0 / (XS * WS)
AF = mybir.ActivationFunctionType
OP = mybir.AluOpType
DR = mybir.MatmulPerfMode.DoubleRow

_PROGRAM = None


def _build_program():
    nc = bacc.Bacc("TRN2", target_bir_lowering=False, debug=False)
    xT = nc.dram_tensor("xT", [I, T], BF16, kind="ExternalInput").ap()
    x8 = nc.dram_tensor("x8", [I, T], FP8, kind="ExternalInput").ap()
    wzT = nc.dram_tensor("wzT", [I, H], FP8, kind="ExternalInput").ap()
    whT = nc.dram_tensor("whT", [I, H], BF16, kind="ExternalInput").ap()
    nbz = nc.dram_tensor("nbz", [H], F32, kind="ExternalInput").ap()   # -b_z
    bh = nc.dram_tensor("bh", [H], F32, kind="ExternalInput").ap()
    h0 = nc.dram_tensor("h0", [H], F32, kind="ExternalInput").ap()
    out = nc.dram_tensor("out", [H, T], BF16, kind="ExternalOutput").ap()

    with tile.TileContext(nc) as tc, ExitStack() as ctx:
        const = ctx.enter_context(tc.tile_pool(name="const", bufs=1))
        xpool = ctx.enter_context(tc.tile_pool(name="xp", bufs=2))
        psum = ctx.enter_context(tc.tile_pool(name="ps", bufs=2, space="PSUM"))
        apool = ctx.enter_context(tc.tile_pool(name="ap", bufs=16))
        act = ctx.enter_context(tc.tile_pool(name="actp", bufs=4))
        hpool = ctx.enter_context(tc.tile_pool(name="hp", bufs=2))

        wzT_r = wzT.rearrange("(k p) h -> p k h", p=P)
        whT_r = whT.rearrange("(n p) h -> p n h", p=P)
        xT_r = xT.rearrange("(n p) t -> p n t", p=P)
        x8_r = x8.rearrange("(k p) t -> p k t", p=P)

        x8_tiles = [[None] * NQ for _ in range(NT)]
        x16_tiles = [[None] * NI for _ in range(NT)]
        h_tiles = [[None] * NH for _ in range(NT)]

        # PE warmup: ~4us of throwaway matmuls on memset tiles right after
        # the preamble, while the PE would otherwise idle waiting for DMA.
        # The HAM clock-gate needs ~3.4us of sustained PE activity to go
        # 4/8 -> 8/8; warming here lets the first real matmuls run at
        # 2.4 GHz instead of 1.2.
        warm_w = const.tile([P, P], BF16, tag="warmw", name="warm_w")
        warm_x = const.tile([P, MN], BF16, tag="warmx", name="warm_x")
        nc.vector.memset(warm_w[:], 0.0)
        nc.vector.memset(warm_x[:], 0.0)
        warm_ps = psum.tile([P, TC], F32, tag="kz", name="warm_ps")
        for k in range(10):
            nc.tensor.matmul(
                warm_ps[:, 0:MN], warm_w[:], warm_x[:], start=True, stop=True
            )

        nbz_sb = const.tile([P, NH], F32, tag="nbz", name="nbz_sb")
        bh_sb = const.tile([P, NH], F32, tag="bh", name="bh_sb")
        h0_sb = const.tile([P, NH], F32, tag="h0", name="h0_sb")
        # Host pre-permutes these to partition-major so each partition reads
        # one contiguous 32B segment (instead of NH 4-byte descriptors).
        nc.scalar.dma_start(nbz_sb[:], nbz.rearrange("(p n) -> p n", n=NH))
        nc.scalar.dma_start(bh_sb[:], bh.rearrange("(p n) -> p n", n=NH))
        nc.scalar.dma_start(h0_sb[:], h0.rearrange("(p n) -> p n", n=NH))

        # g(h_0) -> scan seed [P, NH]; column j seeds channel block j.
        s0 = const.tile([P, NH], F32, tag="s0", name="s0")
        r0 = const.tile([P, NH], F32, tag="r0", name="r0")
        carry = const.tile([P, NH], F32, tag="carry", name="carry")
        nc.scalar.activation(s0[:], h0_sb[:], AF.Sigmoid)
        nc.scalar.activation(r0[:], h0_sb[:], AF.Relu)
        nc.vector.scalar_tensor_tensor(
            carry[:], s0[:], 0.5, r0[:], op0=OP.min, op1=OP.add
        )

        # fp8 side first: wz pair q interleaved with x8 chunk-0 pair q, so
        # the first k_z matmul waits on only 2 gpsimd-ring DMAs.
        wz_sb = []
        for q in range(NQ):
            wz_q = const.tile([P, 2, H], FP8, tag=f"wz{q}", name=f"wz_sb{q}")
            nc.gpsimd.dma_start(wz_q[:], wzT_r[:, 2 * q:2 * q + 2, :])
            x0_q = xpool.tile([P, 2, CHUNKS[0]], FP8, tag=f"x8q{q}",
                              name=f"x8_0_{q}")
            nc.gpsimd.dma_start(x0_q[:], x8_r[:, 2 * q:2 * q + 2, 0:CHUNKS[0]])
            x8_tiles[0][q] = x0_q
            wz_sb.append(wz_q)

        def mm_kz(t, j, tcn):
            kz = psum.tile([P, TC], F32, tag="kz", name=f"kz_{t}_{j}")[:, 0:tcn]
            for q in range(NQ):
                for m0 in range(0, tcn, MN):
                    m1 = min(m0 + MN, tcn)
                    nc.tensor.matmul(
                        kz[:, m0:m1],
                        wz_sb[q][:, :, j * P:(j + 1) * P],
                        x8_tiles[t][q][:, :, m0:m1],
                        start=(q == 0),
                        stop=(q == NQ - 1),
                        perf_mode=DR,
                    )
            return kz

        def mm_kh(t, j, tcn):
            kh = psum.tile([P, TC], F32, tag="kh", name=f"kh_{t}_{j}")[:, 0:tcn]
            for i in range(NI):
                for m0 in range(0, tcn, MN):
                    m1 = min(m0 + MN, tcn)
                    nc.tensor.matmul(
                        kh[:, m0:m1],
                        wh_sb[i][:, j * P:(j + 1) * P],
                        x16_tiles[t][i][:, m0:m1],
                        start=(i == 0),
                        stop=(i == NI - 1),
                    )
            return kh

        def act_a(t, j, tcn, kz):
            a_t = apool.tile([P, TC], BF16, tag="a", name=f"a_{t}_{j}")[:, 0:tcn]
            nc.scalar.activation(
                a_t[:], kz[:], AF.Sigmoid, bias=nbz_sb[:, j:j + 1],
                scale=-DESCALE,
            )
            return a_t

        def tail_chain(t, j, tcn, off, a_t, kh):
            s_t = act.tile([P, TC], BF16, tag="s", name=f"s_{t}_{j}")[:, 0:tcn]
            r_t = act.tile([P, TC], BF16, tag="r", name=f"r_{t}_{j}")[:, 0:tcn]
            gm_t = act.tile([P, TC], BF16, tag="gm", name=f"gm_{t}_{j}")[:, 0:tcn]
            z_t = act.tile([P, TC], BF16, tag="z", name=f"z_{t}_{j}")[:, 0:tcn]
            g_t = act.tile([P, TC], BF16, tag="g", name=f"g_{t}_{j}")[:, 0:tcn]
            v_t = act.tile([P, TC], BF16, tag="v", name=f"v_{t}_{j}")[:, 0:tcn]
            nc.scalar.activation(
                s_t[:], kh[:], AF.Sigmoid, bias=bh_sb[:, j:j + 1], scale=1.0
            )
            nc.scalar.activation(
                r_t[:], kh[:], AF.Relu, bias=bh_sb[:, j:j + 1], scale=1.0
            )
            nc.vector.tensor_scalar_min(gm_t[:], s_t[:], 0.5)
            nc.vector.tensor_scalar(
                z_t[:], a_t[:], 1.0, -1.0, op0=OP.subtract, op1=OP.mult
            )
            nc.vector.tensor_tensor(g_t[:], gm_t[:], r_t[:], op=OP.add)
            nc.vector.tensor_tensor(v_t[:], z_t[:], g_t[:], op=OP.mult)
            h_t = hpool.tile([P, TC], BF16, tag=f"h{j}", name=f"h_{t}_{j}")[:, 0:tcn]
            h_tiles[t][j] = h_t
            init = carry[:, j:j + 1] if t == 0 else (
                h_tiles[t - 1][j][:, CHUNKS[t - 1] - 1:CHUNKS[t - 1]]
            )
            st = nc.sync if j % 2 == 0 else nc.gpsimd
            if t + 1 < NT:
                nc.vector.tensor_tensor_scan(
                    h_t[:], a_t[:], v_t[:], init, op0=OP.mult, op1=OP.add
                )
                st.dma_start(out[j * P:(j + 1) * P, off:off + tcn], h_t[:])
            else:
                # Last chunk: split scan + store in half so the first half's
                # DMA overlaps the second half's scan.
                hm = tcn // 2
                nc.vector.tensor_tensor_scan(
                    h_t[:, 0:hm], a_t[:, 0:hm], v_t[:, 0:hm],
                    init, op0=OP.mult, op1=OP.add
                )
                st.dma_start(
                    out[j * P:(j + 1) * P, off:off + hm], h_t[:, 0:hm]
                )
                nc.vector.tensor_tensor_scan(
                    h_t[:, hm:tcn], a_t[:, hm:tcn], v_t[:, hm:tcn],
                    h_t[:, hm - 1:hm], op0=OP.mult, op1=OP.add
                )
                st.dma_start(
                    out[j * P:(j + 1) * P, off + hm:off + tcn], h_t[:, hm:tcn]
                )

        def prefetch(t):
            if t + 1 >= NT:
                return
            noff = sum(CHUNKS[:t + 1])
            tcn1 = CHUNKS[t + 1]
            for q in range(NQ):
                xn_q = xpool.tile([P, 2, tcn1], FP8, tag=f"x8q{q}",
                                  name=f"x8_{t + 1}_{q}")
                nc.gpsimd.dma_start(
                    xn_q[:], x8_r[:, 2 * q:2 * q + 2, noff:noff + tcn1]
                )
                x8_tiles[t + 1][q] = xn_q
            for i in range(NI):
                xn_i = xpool.tile([P, tcn1], BF16, tag=f"x{i}",
                                  name=f"x_{t + 1}_{i}")
                nc.sync.dma_start(xn_i[:], xT_r[:, i, noff:noff + tcn1])
                x16_tiles[t + 1][i] = xn_i

        # ---- chunk 0: wh + x16 queued, then per-j processing ----
        tcn0 = CHUNKS[0]
        wh_sb = []
        for i in range(NI):
            wh_i = const.tile([P, H], BF16, tag=f"wh{i}", name=f"wh_sb{i}")
            nc.sync.dma_start(wh_i[:], whT_r[:, i, :])
            wh_sb.append(wh_i)
        for i in range(NI):
            x0_i = xpool.tile([P, tcn0], BF16, tag=f"x{i}", name=f"x_0_{i}")
            nc.sync.dma_start(x0_i[:], xT_r[:, i, 0:tcn0])
            x16_tiles[0][i] = x0_i
        a0 = [None] * NH
        for j in range(NH):
            kz0 = mm_kz(0, j, tcn0)
            a0[j] = act_a(0, j, tcn0, kz0)
        # Pin chunk-0's k_h work to sim-time >= 14us so the scheduler keeps
        # every k_z matmul (fed by 1.5 MB of fp8 DMA) ahead of the k_h
        # matmuls (which need all 4.5 MB of wh + x16 + fp8 data) in the PE
        # FIFO; otherwise its PSUM-group striping head-of-line-blocks the
        # queue on the wh/x16 loads.
        with tc.tile_wait_until(0.014):
            for j in range(NH):
                kh = mm_kh(0, j, tcn0)
                tail_chain(0, j, tcn0, 0, a0[j], kh)
        prefetch(0)

        off = tcn0
        for t in range(1, NT):
            tcn = CHUNKS[t]
            for j in range(NH):
                kz = mm_kz(t, j, tcn)
                a_t = act_a(t, j, tcn, kz)
                kh = mm_kh(t, j, tcn)
                tail_chain(t, j, tcn, off, a_t, kh)
            prefetch(t)
            off += tcn

    nc.compile()
    return nc


def _get_program():
    global _PROGRAM
    if _PROGRAM is None:
        _PROGRAM = _build_program()
    return _PROGRAM


def _make_in_maps(x, h_0, W_z, b_z, W_h, b_h):
    def pmajor(v):
        # [NH*P] channel-major -> partition-major so the SBUF-side [P, NH]
        # tile DMA reads one contiguous segment per partition.
        return np.ascontiguousarray(
            v.astype(np.float32).reshape(NH, P).T.reshape(-1)
        )

    wzT = np.ascontiguousarray((W_z.T * WS).astype(FP8_NP))
    whT = np.ascontiguousarray(W_h.T.astype(BF16_NP))
    nbz = pmajor(-b_z)
    bh = pmajor(b_h)
    in_maps = []
    for b in range(B):
        xTb = x[b].T
        in_maps.append({
            "xT": np.ascontiguousarray(xTb.astype(BF16_NP)),
            "x8": np.ascontiguousarray((xTb * XS).astype(FP8_NP)),
            "wzT": wzT,
            "whT": whT,
            "nbz": nbz,
            "bh": bh,
            "h0": pmajor(h_0[b]),
        })
    return in_maps


def _run(x, h_0, W_z, b_z, W_h, b_h, trace=False):
    x, h_0, W_z, b_z, W_h, b_h = (
        np.asarray(a) for a in (x, h_0, W_z, b_z, W_h, b_h)
    )
    nc = _get_program()
    in_maps = _make_in_maps(x, h_0, W_z, b_z, W_h, b_h)
    res = run_bass_kernel_spmd(nc, in_maps, core_ids=list(range(B)), trace=trace)
    out = np.stack(
        [res.results[b]["out"].T.astype(np.float32) for b in range(B)], axis=0
    )
    return out, res


def kernel(x, h_0, W_z, b_z, W_h, b_h):
    out, _ = _run(x, h_0, W_z, b_z, W_h, b_h)
    return out


# revision 25
# speedup vs baseline: 1.0214x; 1.0214x over previous
"""MinGRU on Trainium2 (Bass/Tile), data-parallel over batch on 8 NeuronCores.

Math (per batch element, per hidden channel):
    k_z = x @ W_z.T + b_z
    k_h = x @ W_h.T + b_h
    a   = sigmoid(-k_z)                  # = exp(log_coeffs) in the reference
    z   = sigmoid(k_z) = 1 - a
    g(u)= u + 0.5 if u >= 0 else sigmoid(u)
    v   = z * g(k_h)                     # = exp(log_values[1:])
    h_t = a_t * h_{t-1} + v_t,  h_init = g(h_0)        (t = 1..T)
Output is h_1..h_T, shape [B, T, H].

Device layout: each core gets one batch element. Hidden dim H lives on
SBUF partitions (8 tiles of 128), time T on the free dim, so the
recurrence maps to the DVE TensorTensorScan instruction (fp32 state).

Precision/throughput split (validated by CPU emulation, rel_err 1.5e-2
vs 2e-2 gate):
  - k_z runs in fp8(e4m3) with DoubleRow perf mode (2 contraction rows
    per PE cell -> 2x matmul rate). Errors in k_z only reach h through
    sigmoid' <= 1/4, so fp8 noise is tolerable there.
  - k_h runs in bf16 (g has slope 1; fp8 here fails the error gate).
  - Post-PSUM work uses bf16 tiles and only tensor_scalar (4x DVE rate)
    / tensor_tensor (2x) / scan ops; the 1x-rate STT form is avoided:
        gm = min(s, 0.5); g = gm + r; z = (a-1)*(-1); v = z*g
        h = scan(a, v): state = a*state + v
    Each chunk's scan seeds directly from the previous h tile (no carry
    copies). fp8 operands are pre-scaled (x*32, W_z*4096); the 2^-17
    descale folds into the ACT scale. Output is bf16, upcast on host.

DMA rings: gpsimd carries wz8 + fp8 x chunks, scalar carries consts +
bf16 x chunks, sync carries wh (queued before any store) + the output
stores. Stores wait on the scan, so no load may queue behind them.
DMA-completion semaphore slots rotate per queue and only the FIRST
queue-consumer of a DMA gets a wait instruction, so each chunk's
loads are emitted immediately before their first consumers (chunk
boundaries). Chunk 0 runs all 8 k_z PE groups (which need only 1.5 MB
of fp8 data) before any k_h group so the PE starts ~7us in while
wh/x16 stream.
"""

import numpy as np
from contextlib import ExitStack

import concourse.bass as bass
import concourse.tile as tile
from concourse import bacc, mybir
from concourse.bass_utils import run_bass_kernel_spmd

B, T, I, H = 8, 4096, 1024, 1024
P = 128           # SBUF partitions
TC = 1024         # max T chunk for the PSUM/ACT/DVE stage (2 PSUM banks)
MN = 512          # matmul moving free dim (one PSUM bank of fp32)
CHUNKS = [512, 1024, 1024, 1024, 512]
assert sum(CHUNKS) == T
NI, NH = I // P, H // P
NQ = NI // 2      # fp8 DoubleRow contraction pairs
NT = len(CHUNKS)
F32 = mybir.dt.float32
BF16 = mybir.dt.bfloat16
FP8 = mybir.dt.float8e4
import ml_dtypes
BF16_NP = ml_dtypes.bfloat16
FP8_NP = ml_dtypes.float8_e4m3
XS = 32.0         # host-side fp8 scale on x
WS = 4096.0       # host-side fp8 scale on W_z
DESCALE = 1.0 / (XS * WS)
AF = mybir.ActivationFunctionType
OP = mybir.AluOpType
DR = mybir.MatmulPerfMode.DoubleRow

_PROGRAM = None


def _build_program():
    nc = bacc.Bacc("TRN2", target_bir_lowering=False, debug=False)
    xT = nc.dram_tensor("xT", [I, T], BF16, kind="ExternalInput").ap()
    x8 = nc.dram_tensor("x8", [I, T], FP8, kind="ExternalInput").ap()
    wzT = nc.dram_tensor("wzT", [I, H], FP8, kind="ExternalInput").ap()
    whT = nc.dram_tensor("whT", [I, H], BF16, kind="ExternalInput").ap()
    nbz = nc.dram_tensor("nbz", [H], F32, kind="ExternalInput").ap()   # -b_z
    bh = nc.dram_tensor("bh", [H], F32, kind="ExternalInput").ap()
    h0 = nc.dram_tensor("h0", [H], F32, kind="ExternalInput").ap()
    out = nc.dram_tensor("out", [H, T], BF16, kind="ExternalOutput").ap()

    with tile.TileContext(nc) as tc, ExitStack() as ctx:
        const = ctx.enter_context(tc.tile_pool(name="const", bufs=1))
        xpool = ctx.enter_context(tc.tile_pool(name="xp", bufs=2))
        psum = ctx.enter_context(tc.tile_pool(name="ps", bufs=2, space="PSUM"))
        apool = ctx.enter_context(tc.tile_pool(name="ap", bufs=16))
        act = ctx.enter_context(tc.tile_pool(name="actp", bufs=4))
        hpool = ctx.enter_context(tc.tile_pool(name="hp", bufs=2))

        wzT_r = wzT.rearrange("(k p) h -> p k h", p=P)
        whT_r = whT.rearrange("(n p) h -> p n h", p=P)
        xT_r = xT.rearrange("(n p) t -> p n t", p=P)
        x8_r = x8.rearrange("(k p) t -> p k t", p=P)

        x8_tiles = [[None] * NQ for _ in range(NT)]
        x16_tiles = [[None] * NI for _ in range(NT)]
        h_tiles = [[None] * NH for _ in range(NT)]

        # PE warmup: ~4us of throwaway matmuls on memset tiles right after
        # the preamble, while the PE would otherwise idle waiting for DMA.
        # The HAM clock-gate needs ~3.4us of sustained PE activity to go
        # 4/8 -> 8/8; warming here lets the first real matmuls run at
        # 2.4 GHz instead of 1.2.
        warm_w = const.tile([P, P], BF16, tag="warmw", name="warm_w")
        warm_x = const.tile([P, MN], BF16, tag="warmx", name="warm_x")
        nc.vector.memset(warm_w[:], 0.0)
        nc.vector.memset(warm_x[:], 0.0)
        warm_ps = psum.tile([P, TC], F32, tag="kz", name="warm_ps")
        for k in range(10):
            nc.tensor.matmul(
                warm_ps[:, 0:MN], warm_w[:], warm_x[:], start=True, stop=True
            )

        nbz_sb = const.tile([P, NH], F32, tag="nbz", name="nbz_sb")
        bh_sb = const.tile([P, NH], F32, tag="bh", name="bh_sb")
        h0_sb = const.tile([P, NH], F32, tag="h0", name="h0_sb")
        # Host pre-permutes these to partition-major so each partition reads
        # one contiguous 32B segment (instead of NH 4-byte descriptors).
        nc.scalar.dma_start(nbz_sb[:], nbz.rearrange("(p n) -> p n", n=NH))
        nc.scalar.dma_start(bh_sb[:], bh.rearrange("(p n) -> p n", n=NH))
        nc.scalar.dma_start(h0_sb[:], h0.rearrange("(p n) -> p n", n=NH))

        # g(h_0) -> scan seed [P, NH]; column j seeds channel block j.
        s0 = const.tile([P, NH], F32, tag="s0", name="s0")
        r0 = const.tile([P, NH], F32, tag="r0", name="r0")
        carry = const.tile([P, NH], F32, tag="carry", name="carry")
        nc.scalar.activation(s0[:], h0_sb[:], AF.Sigmoid)
        nc.scalar.activation(r0[:], h0_sb[:], AF.Relu)
        nc.vector.scalar_tensor_tensor(
            carry[:], s0[:], 0.5, r0[:], op0=OP.min, op1=OP.add
        )

        # fp8 side first: wz pair q interleaved with x8 chunk-0 pair q, so
        # the first k_z matmul waits on only 2 gpsimd-ring DMAs.
        wz_sb = []
        for q in range(NQ):
            wz_q = const.tile([P, 2, H], FP8, tag=f"wz{q}", name=f"wz_sb{q}")
            nc.gpsimd.dma_start(wz_q[:], wzT_r[:, 2 * q:2 * q + 2, :])
            x0_q = xpool.tile([P, 2, CHUNKS[0]], FP8, tag=f"x8q{q}",
                              name=f"x8_0_{q}")
            nc.gpsimd.dma_start(x0_q[:], x8_r[:, 2 * q:2 * q + 2, 0:CHUNKS[0]])
            x8_tiles[0][q] = x0_q
            wz_sb.append(wz_q)

        def mm_kz(t, j, tcn):
            kz = psum.tile([P, TC], F32, tag="kz", name=f"kz_{t}_{j}")[:, 0:tcn]
            for q in range(NQ):
                for m0 in range(0, tcn, MN):
                    m1 = min(m0 + MN, tcn)
                    nc.tensor.matmul(
                        kz[:, m0:m1],
                        wz_sb[q][:, :, j * P:(j + 1) * P],
                        x8_tiles[t][q][:, :, m0:m1],
                        start=(q == 0),
                        stop=(q == NQ - 1),
                        perf_mode=DR,
                    )
            return kz

        def mm_kh(t, j, tcn):
            kh = psum.tile([P, TC], F32, tag="kh", name=f"kh_{t}_{j}")[:, 0:tcn]
            for i in range(NI):
                for m0 in range(0, tcn, MN):
                    m1 = min(m0 + MN, tcn)
                    nc.tensor.matmul(
                        kh[:, m0:m1],
                        wh_sb[i][:, j * P:(j + 1) * P],
                        x16_tiles[t][i][:, m0:m1],
                        start=(i == 0),
                        stop=(i == NI - 1),
                    )
            return kh

        def act_a(t, j, tcn, kz):
            a_t = apool.tile([P, TC], BF16, tag="a", name=f"a_{t}_{j}")[:, 0:tcn]
            nc.scalar.activation(
                a_t[:], kz[:], AF.Sigmoid, bias=nbz_sb[:, j:j + 1],
                scale=-DESCALE,
            )
            return a_t

        def tail_chain(t, j, tcn, off, a_t, kh):
            s_t = act.tile([P, TC], BF16, tag="s", name=f"s_{t}_{j}")[:, 0:tcn]
            r_t = act.tile([P, TC], BF16, tag="r", name=f"r_{t}_{j}")[:, 0:tcn]
            gm_t = act.tile([P, TC], BF16, tag="gm", name=f"gm_{t}_{j}")[:, 0:tcn]
            z_t = act.tile([P, TC], BF16, tag="z", name=f"z_{t}_{j}")[:, 0:tcn]
            g_t = act.tile([P, TC], BF16, tag="g", name=f"g_{t}_{j}")[:, 0:tcn]
            v_t = act.tile([P, TC], BF16, tag="v", name=f"v_{t}_{j}")[:, 0:tcn]
            nc.scalar.activation(
                s_t[:], kh[:], AF.Sigmoid, bias=bh_sb[:, j:j + 1], scale=1.0
            )
            nc.scalar.activation(
                r_t[:], kh[:], AF.Relu, bias=bh_sb[:, j:j + 1], scale=1.0
            )
            nc.vector.tensor_scalar_min(gm_t[:], s_t[:], 0.5)
            nc.vector.tensor_scalar(
                z_t[:], a_t[:], 1.0, -1.0, op0=OP.subtract, op1=OP.mult
            )
            nc.vector.tensor_tensor(g_t[:], gm_t[:], r_t[:], op=OP.add)
            nc.vector.tensor_tensor(v_t[:], z_t[:], g_t[:], op=OP.mult)
            h_t = hpool.tile([P, TC], BF16, tag=f"h{j}", name=f"h_{t}_{j}")[:, 0:tcn]
            h_tiles[t][j] = h_t
            init = carry[:, j:j + 1] if t == 0 else (
                h_tiles[t - 1][j][:, CHUNKS[t - 1] - 1:CHUNKS[t - 1]]
            )
            st = nc.sync if j % 2 == 0 else nc.gpsimd
            if t + 1 < NT:
                nc.vector.tensor_tensor_scan(
                    h_t[:], a_t[:], v_t[:], init, op0=OP.mult, op1=OP.add
                )
                st.dma_start(out[j * P:(j + 1) * P, off:off + tcn], h_t[:])
            else:
                # Last chunk: split scan + store in half so the first half's
                # DMA overlaps the second half's scan.
                hm = tcn // 2
                nc.vector.tensor_tensor_scan(
                    h_t[:, 0:hm], a_t[:, 0:hm], v_t[:, 0:hm],
                    init, op0=OP.mult, op1=OP.add
                )
                st.dma_start(
                    out[j * P:(j + 1) * P, off:off + hm], h_t[:, 0:hm]
                )
                nc.vector.tensor_tensor_scan(
                    h_t[:, hm:tcn], a_t[:, hm:tcn], v_t[:, hm:tcn],
                    h_t[:, hm - 1:hm], op0=OP.mult, op1=OP.add
                )
                st.dma_start(
                    out[j * P:(j + 1) * P, off + hm:off + tcn], h_t[:, hm:tcn]
                )

        def prefetch(t):
            if t + 1 >= NT:
                return
            noff = sum(CHUNKS[:t + 1])
            tcn1 = CHUNKS[t + 1]
            for q in range(NQ):
                xn_q = xpool.tile([P, 2, tcn1], FP8, tag=f"x8q{q}",
                                  name=f"x8_{t + 1}_{q}")
                nc.gpsimd.dma_start(
                    xn_q[:], x8_r[:, 2 * q:2 * q + 2, noff:noff + tcn1]
                )
                x8_tiles[t + 1][q] = xn_q
            for i in range(NI):
                xn_i = xpool.tile([P, tcn1], BF16, tag=f"x{i}",
                                  name=f"x_{t + 1}_{i}")
                nc.sync.dma_start(xn_i[:], xT_r[:, i, noff:noff + tcn1])
                x16_tiles[t + 1][i] = xn_i

        # ---- chunk 0: wh + x16 queued, then per-j processing ----
        tcn0 = CHUNKS[0]
        wh_sb = []
        for i in range(NI):
            wh_i = const.tile([P, H], BF16, tag=f"wh{i}", name=f"wh_sb{i}")
            nc.sync.dma_start(wh_i[:], whT_r[:, i, :])
            wh_sb.append(wh_i)
        for i in range(NI):
            x0_i = xpool.tile([P, tcn0], BF16, tag=f"x{i}", name=f"x_0_{i}")
            nc.sync.dma_start(x0_i[:], xT_r[:, i, 0:tcn0])
            x16_tiles[0][i] = x0_i
        a0 = [None] * NH
        for j in range(NH):
            kz0 = mm_kz(0, j, tcn0)
            a0[j] = act_a(0, j, tcn0, kz0)
        # Pin chunk-0's k_h work to sim-time >= 14us so the scheduler keeps
        # every k_z matmul (fed by 1.5 MB of fp8 DMA) ahead of the k_h
        # matmuls (which need all 4.5 MB of wh + x16 + fp8 data) in the PE
        # FIFO; otherwise its PSUM-group striping head-of-line-blocks the
        # queue on the wh/x16 loads.
        with tc.tile_wait_until(0.014):
            for j in range(NH):
                kh = mm_kh(0, j, tcn0)
                tail_chain(0, j, tcn0, 0, a0[j], kh)
        prefetch(0)

        off = tcn0
        for t in range(1, NT):
            tcn = CHUNKS[t]
            for j in range(NH):
                kz = mm_kz(t, j, tcn)
                a_t = act_a(t, j, tcn, kz)
                kh = mm_kh(t, j, tcn)
                tail_chain(t, j, tcn, off, a_t, kh)
            prefetch(t)
            off += tcn

    nc.compile()
    return nc


def _get_program():
    global _PROGRAM
    if _PROGRAM is None:
        _PROGRAM = _build_program()
    return _PROGRAM


def _make_in_maps(x, h_0, W_z, b_z, W_h, b_h):
    def pmajor(v):
        # [NH*P] channel-major -> partition-major so the SBUF-side [P, NH]
        # tile DMA reads one contiguous segment per partition.
        return np.ascontiguousarray(
            v.astype(np.float32).reshape(NH, P).T.reshape(-1)
        )

    wzT = np.ascontiguousarray((W_z.T * WS).astype(FP8_NP))
    whT = np.ascontiguousarray(W_h.T.astype(BF16_NP))
    nbz = pmajor(-b_z)
    bh = pmajor(b_h)
    in_maps = []
    for b in range(B):
        xTb = x[b].T
        in_maps.append({
            "xT": np.ascontiguousarray(xTb.astype(BF16_NP)),
            "x8": np.ascontiguousarray((xTb * XS).astype(FP8_NP)),
            "wzT": wzT,
            "whT": whT,
            "nbz": nbz,
            "bh": bh,
            "h0": pmajor(h_0[b]),
        })
    return in_maps


def _run(x, h_0, W_z, b_z, W_h, b_h, trace=False):
    x, h_0, W_z, b_z, W_h, b_h = (
        np.asarray(a) for a in (x, h_0, W_z, b_z, W_h, b_h)
    )
    nc = _get_program()
    in_maps = _make_in_maps(x, h_0, W_z, b_z, W_h, b_h)
    res = run_bass_kernel_spmd(nc, in_maps, core_ids=list(range(B)), trace=trace)
    out = np.stack(
        [res.results[b]["out"].T.astype(np.float32) for b in range(B)], axis=0
    )
    return out, res


def kernel(x, h_0, W_z, b_z, W_h, b_h):
    out, _ = _run(x, h_0, W_z, b_z, W_h, b_h)
    return out
